# revision 1
# baseline (speedup 1.0000x reference)
"""Trainium2 Bass kernel for nn_AttentionBlock (GroupNorm + attention block),
data-parallel over batch across 8 NeuronCores.

Reference computation (per batch element b, C=512, N=H*W=1024, heads=8, hd=64):
  xn   = GroupNorm32(x) * gamma + beta
  qkv  = w_qkv @ xn + b_qkv        (1x1 conv == matmul over channels)
  attn = softmax(q^T k / sqrt(hd)) ; ha = attn @ v ; out = x + w_proj @ ha + b_proj

Sharding: batch B=8 -> one batch element per core. No collectives.

Per-core dataflow (matmuls in bf16 with f32 PSUM accumulation; weights are
passed as f32 and converted to bf16 on chip):
  - GroupNorm stats in f32: per-channel sum / sum-of-squares on DVE, group
    reduction + per-channel re-broadcast via tiny TensorE matmuls,
    rsqrt(var+eps) computed as exp(-0.5*ln(var+eps)) to stay in a single
    ScalarE table set (with the softmax Exp).
  - q,k produced in [channel, n] layout with host-pretransposed w_qkT.
  - v produced directly TRANSPOSED, v_T[n, c_v] = xn^T @ w_v^T (host
    pretransposed rhs), so attention needs no on-chip transposes. The v bias
    is folded in via a ones-row matmul accumulation. v_T is stored with
    head stride 65: 64 v columns + a ones column per head.
  - Scores computed transposed: S_T[m, n] = k_h^T q_h (K = hd = 64).
    Softmax without max subtraction (|scores*0.125| <~ 8, exp is safe in f32):
    P_T = exp(S_T * 0.125) on ScalarE directly out of PSUM (doubles as the
    PSUM eviction), written as bf16.
  - AV: ha_u[65, n] = [v_T | ones]^T @ P_T accumulated over the 8 m-chunks;
    row 64 is the softmax denominator Z. Normalization by 1/Z is applied at
    eviction: reciprocal on DVE, broadcast over partitions via a K=1 ones
    matmul on TensorE, multiply on DVE (writing bf16 for the proj matmul).
  - proj: out = (psum + b_proj) + x fused in one DVE scalar_tensor_tensor.
"""

import os

import numpy as np

import concourse.bass as bass
import concourse.bacc as bacc
import concourse.mybir as mybir
import concourse.tile as tile
from concourse.bass_utils import run_bass_kernel_spmd

F32 = mybir.dt.float32
BF16 = mybir.dt.bfloat16
AF = mybir.ActivationFunctionType
ALU = mybir.AluOpType

B = 8
C = 512
N = 1024          # H*W = 32*32
H = 8             # num heads
HD = 64           # head dim
G = 32            # groups
GS = C // G       # channels per group = 16
CCH = 4           # channel chunks of 128
NT = 2            # n tiles of 512
MT = 8            # m tiles of 128
EPS = 1e-5
P = 128
NCORES = 8

_CACHE = {}


def build_nc():
    nc = bacc.Bacc(
        "TRN2", target_bir_lowering=False, debug=False, num_devices=NCORES
    )

    # All parameters are 2-D float32, pre-arranged on the host so every DMA
    # below is a plain contiguous copy.
    x_d = nc.declare_dram_parameter("x", [C, N], F32, isOutput=False)
    wqk_d = nc.declare_dram_parameter("w_qkT", [C, 2 * C], F32, isOutput=False)
    bqk_d = nc.declare_dram_parameter("b_qk", [P, 8], F32, isOutput=False)
    wv_d = nc.declare_dram_parameter("w_vT", [C + 1, C], F32, isOutput=False)
    wp_d = nc.declare_dram_parameter("w_pT", [C, C], F32, isOutput=False)
    bp_d = nc.declare_dram_parameter("b_p", [P, CCH], F32, isOutput=False)
    gm_d = nc.declare_dram_parameter("gamma", [P, CCH], F32, isOutput=False)
    bt_d = nc.declare_dram_parameter("beta", [P, CCH], F32, isOutput=False)
    gsel_d = nc.declare_dram_parameter("gsel", [P, 8], F32, isOutput=False)
    gselT_d = nc.declare_dram_parameter("gselT", [8, P], F32, isOutput=False)
    out_d = nc.declare_dram_parameter("out", [C, N], F32, isOutput=True)

    with tile.TileContext(nc) as tc:
        with (
            tc.tile_pool(name="singles", bufs=1) as singles,
            tc.tile_pool(name="wstage", bufs=2) as wstage,
            tc.tile_pool(name="ps", bufs=2, space="PSUM") as ps_pool,
            tc.tile_pool(name="ps_av", bufs=2, space="PSUM") as ps_av_pool,
        ):
            # ---------------- static tiles ----------------
            x_sb = singles.tile([P, CCH, N], F32)
            wqk_sb = singles.tile([P, CCH, 2 * C], BF16)
            bqk_sb = singles.tile([P, 8], F32)
            wv_sb = singles.tile([P, CCH, C], BF16)
            wvb_sb = singles.tile([1, C], BF16)
            wp_sb = singles.tile([P, CCH, C], BF16)
            bp_sb = singles.tile([P, CCH], F32)
            gm_sb = singles.tile([P, CCH], F32)
            bt_sb = singles.tile([P, CCH], F32)
            gsel_sb = singles.tile([P, 8], BF16)
            gselT_sb = singles.tile([8, P], BF16)
            gsel_st = singles.tile([P, 8], F32)
            gselT_st = singles.tile([8, P], F32)
            s12_bf = singles.tile([P, 8], BF16)
            mu_rs_bf = singles.tile([8, 8], BF16)
            ones_row = singles.tile([1, P], BF16)
            ones64 = singles.tile([1, HD], BF16)

            xn_sb = singles.tile([P, CCH, N], BF16)
            qk_sb = singles.tile([P, 8, N], BF16)       # ot 0-3: q, 4-7: k
            vT_sb = singles.tile([P, MT, H * (HD + 16)], BF16)
            pT_a = singles.tile([P, MT, N], BF16)
            pT_b = singles.tile([P, MT, N], BF16)
            ha_sb = singles.tile([P, CCH, N], BF16)
            out_sb = singles.tile([P, CCH, N], F32)

            s12_sb = singles.tile([P, 8], F32)          # cols 0-3 sum, 4-7 sumsq
            sq_scr = singles.tile([P, N], F32)
            mu_rs = singles.tile([8, 8], F32)           # cols 0-3 mu, 4-7 rs
            ex2_sb = singles.tile([8, CCH], F32)
            tmp8 = singles.tile([8, CCH], F32)
            var_sb = singles.tile([8, CCH], F32)
            lnv_sb = singles.tile([8, CCH], F32)
            s0_sb = singles.tile([P, CCH], F32)
            sbias_sb = singles.tile([P, CCH], F32)
            tmp128 = singles.tile([P, CCH], F32)
            eps_sb = singles.tile([8, 1], F32)
            zinv_t = [
                singles.tile([16, N], BF16, name=f"zinv{i}") for i in range(2)
            ]
            zln_t = [
                singles.tile([16, N], F32, name=f"zln{i}") for i in range(2)
            ]
            ones16 = singles.tile([16, HD], BF16)
            zb_t = [singles.tile([HD, N], F32, name=f"zb{i}") for i in range(2)]

            # selector matrices DMA'd from host, converted to bf16 on chip
            nc.sync.dma_start(gsel_st[:], gsel_d.ap())
            nc.sync.dma_start(gselT_st[:], gselT_d.ap())
            nc.vector.tensor_copy(gsel_sb[:], gsel_st[:])
            nc.vector.tensor_copy(gselT_sb[:], gselT_st[:])
            nc.vector.memset(ones_row[:], 1.0)
            nc.vector.memset(ones64[:], 1.0)
            nc.vector.memset(eps_sb[:], EPS)
            nc.vector.memset(ones16[:], 1.0 / 16.0)

            # ---------------- input DMAs + weight bf16 conversion ----------
            x_v = x_d.ap().rearrange("(cc p) n -> p cc n", p=P)
            for cc in range(CCH):
                nc.sync.dma_start(x_sb[:, cc, :], x_v[:, cc, :])

            wqk_st = wstage.tile([P, CCH, 2 * C], F32, tag="wst")
            nc.sync.dma_start(
                wqk_st[:], wqk_d.ap().rearrange("(cc p) o -> p cc o", p=P)
            )
            nc.vector.tensor_copy(wqk_sb[:], wqk_st[:])

            wv_st = wstage.tile([P, CCH, C], F32, tag="wst")
            nc.sync.dma_start(
                wv_st[:], wv_d.ap()[0:C, :].rearrange("(cc p) v -> p cc v", p=P)
            )
            nc.vector.tensor_copy(wv_sb[:], wv_st[:])
            wvb_st = wstage.tile([1, C], F32, tag="wvbst")
            nc.sync.dma_start(wvb_st[:], wv_d.ap()[C : C + 1, :])
            nc.vector.tensor_copy(wvb_sb[:], wvb_st[:])

            wp_st = wstage.tile([P, CCH, C], F32, tag="wst")
            nc.sync.dma_start(
                wp_st[:], wp_d.ap().rearrange("(cc p) o -> p cc o", p=P)
            )
            nc.vector.tensor_copy(wp_sb[:], wp_st[:])

            nc.sync.dma_start(bqk_sb[:], bqk_d.ap())
            nc.sync.dma_start(bp_sb[:], bp_d.ap())
            nc.sync.dma_start(gm_sb[:], gm_d.ap())
            nc.sync.dma_start(bt_sb[:], bt_d.ap())

            # ---------------- GroupNorm stats ----------------
            for cc in range(CCH):
                nc.vector.reduce_sum(
                    s12_sb[:, cc : cc + 1], x_sb[:, cc, :], axis=mybir.AxisListType.X
                )
                nc.scalar.activation(
                    sq_scr[:], x_sb[:, cc, :], AF.Square,
                    accum_out=s12_sb[:, 4 + cc : 5 + cc],
                )
            # group reduce: [8 groups-in-chunk, 8 (s1 x cc, s2 x cc)]
            nc.vector.tensor_copy(s12_bf[:], s12_sb[:])
            ps_st = ps_pool.tile([P, N], F32, tag="ps")
            nc.tensor.matmul(
                ps_st[0:8, 0:8], gsel_sb[:], s12_bf[:], start=True, stop=True
            )
            inv_cnt = 1.0 / (GS * N)
            nc.vector.tensor_scalar_mul(mu_rs[:, 0:4], ps_st[0:8, 0:4], inv_cnt)
            nc.vector.tensor_scalar_mul(ex2_sb[:], ps_st[0:8, 4:8], inv_cnt)
            nc.vector.tensor_mul(tmp8[:], mu_rs[:, 0:4], mu_rs[:, 0:4])
            nc.vector.tensor_sub(var_sb[:], ex2_sb[:], tmp8[:])
            # rsqrt(var+eps) = exp(-0.5 * ln(var+eps)); keeps one ACT table set
            nc.scalar.activation(lnv_sb[:], var_sb[:], AF.Ln, bias=eps_sb[:])
            nc.scalar.activation(mu_rs[:, 4:8], lnv_sb[:], AF.Exp, scale=-0.5)
            # broadcast group stats back to channels
            nc.vector.tensor_copy(mu_rs_bf[:], mu_rs[:])
            ps_bc = ps_pool.tile([P, N], F32, tag="ps")
            nc.tensor.matmul(
                ps_bc[0:P, 0:8], gselT_sb[:], mu_rs_bf[:], start=True, stop=True
            )
            nc.vector.tensor_mul(s0_sb[:], ps_bc[0:P, 4:8], gm_sb[:])
            nc.vector.tensor_mul(tmp128[:], ps_bc[0:P, 0:4], s0_sb[:])
            nc.vector.tensor_sub(sbias_sb[:], bt_sb[:], tmp128[:])
            # xn = x * s0 + sbias  (bf16)
            for cc in range(CCH):
                nc.vector.tensor_scalar(
                    out=xn_sb[:, cc, :],
                    in0=x_sb[:, cc, :],
                    scalar1=s0_sb[:, cc : cc + 1],
                    scalar2=sbias_sb[:, cc : cc + 1],
                    op0=ALU.mult,
                    op1=ALU.add,
                )

            # ---------------- q, k ----------------
            for ot in range(8):
                ps_qk = ps_pool.tile([P, N], F32, tag="ps")
                for nt in range(NT):
                    for cc in range(CCH):
                        nc.tensor.matmul(
                            ps_qk[:, nt * 512 : (nt + 1) * 512],
                            wqk_sb[:, cc, ot * P : (ot + 1) * P],
                            xn_sb[:, cc, nt * 512 : (nt + 1) * 512],
                            start=(cc == 0),
                            stop=(cc == CCH - 1),
                        )
                nc.scalar.activation(
                    qk_sb[:, ot, :], ps_qk[:], AF.Identity,
                    bias=bqk_sb[:, ot : ot + 1],
                )

            # ---------------- v_T (+ bias via ones row) ----------------
            for mt in range(MT):
                ps_v = ps_pool.tile([P, N], F32, tag="ps")
                for cc in range(CCH):
                    nc.tensor.matmul(
                        ps_v[:, 0:C],
                        xn_sb[:, cc, mt * P : (mt + 1) * P],
                        wv_sb[:, cc, :],
                        start=(cc == 0),
                        stop=False,
                    )
                nc.tensor.matmul(
                    ps_v[:, 0:C], ones_row[:], wvb_sb[:], start=False, stop=True
                )
                nc.vector.tensor_copy(
                    vT_sb[:, mt, :]
                    .rearrange("p (h d) -> p h d", h=H)[:, :, 0:HD],
                    ps_v[:, 0:C].rearrange("p (h d) -> p h d", h=H),
                )
            nc.vector.memset(
                vT_sb[:].rearrange("p mt (h d) -> p mt h d", h=H)[:, :, :, HD : HD + 16],
                1.0,
            )

            # ---------------- attention, software-pipelined over heads ----
            # PE program order: S_T(h) ... AV(h-1) ... so AV's dependency
            # (exp of head h-1 on ScalarE) hides behind S_T(h)'s matmuls.
            def st_exp(h):
                pT = pT_a if h % 2 == 0 else pT_b
                po = (h % 2) * HD
                ot = h // 2
                for mt in range(MT):
                    ps_s = ps_pool.tile([P, N], F32, tag="ps", name=f"ps_s{h}_{mt}")
                    for nt in range(NT):
                        nc.tensor.matmul(
                            ps_s[:, nt * 512 : (nt + 1) * 512],
                            qk_sb[po : po + HD, 4 + ot, mt * P : (mt + 1) * P],
                            qk_sb[po : po + HD, ot, nt * 512 : (nt + 1) * 512],
                            start=True,
                            stop=True,
                        )
                    nc.scalar.activation(
                        pT[:, mt, :], ps_s[:], AF.Exp, scale=float(HD) ** -0.5
                    )

            def av_evict(h):
                pT = pT_a if h % 2 == 0 else pT_b
                po = (h % 2) * HD
                ot = h // 2
                zinv = zinv_t[h % 2]
                zln = zln_t[h % 2]
                zb = zb_t[h % 2]
                W = HD + 16
                ps_av = ps_av_pool.tile([P, N], F32, tag="av", name=f"ps_av{h}")
                for nt in range(NT):
                    for mt in range(MT):
                        nc.tensor.matmul(
                            ps_av[0:W, nt * 512 : (nt + 1) * 512],
                            vT_sb[:, mt, h * W : (h + 1) * W],
                            pT[:, mt, nt * 512 : (nt + 1) * 512],
                            start=(mt == 0),
                            stop=(mt == MT - 1),
                        )
                nc.vector.reciprocal(zln[:], ps_av[HD:W, :])
                nc.vector.tensor_copy(zinv[:], zln[:])
                ps_zb = ps_pool.tile([P, N], F32, tag="ps", name=f"ps_zb{h}")
                for nt in range(NT):
                    nc.tensor.matmul(
                        ps_zb[0:HD, nt * 512 : (nt + 1) * 512],
                        ones16[:],
                        zinv[:, nt * 512 : (nt + 1) * 512],
                        start=True,
                        stop=True,
                    )
                nc.vector.tensor_copy(zb[:], ps_zb[0:HD, :])
                nc.vector.tensor_mul(
                    ha_sb[po : po + HD, ot, :], ps_av[0:HD, :], zb[:]
                )

            st_exp(0)
            for h in range(1, H):
                st_exp(h)
                av_evict(h - 1)
            av_evict(H - 1)

            # ---------------- proj + bias + residual ----------------
            out_v = out_d.ap().rearrange("(ot p) n -> p ot n", p=P)
            for ot in range(CCH):
                ps_p = ps_pool.tile([P, N], F32, tag="ps")
                for nt in range(NT):
                    for cc in range(CCH):
                        nc.tensor.matmul(
                            ps_p[:, nt * 512 : (nt + 1) * 512],
                            wp_sb[:, cc, ot * P : (ot + 1) * P],
                            ha_sb[:, cc, nt * 512 : (nt + 1) * 512],
                            start=(cc == 0),
                            stop=(cc == CCH - 1),
                        )
                nc.vector.scalar_tensor_tensor(
                    out=out_sb[:, ot, :],
                    in0=ps_p[:],
                    scalar=bp_sb[:, ot : ot + 1],
                    in1=x_sb[:, ot, :],
                    op0=ALU.add,
                    op1=ALU.add,
                )
                nc.sync.dma_start(out_v[:, ot, :], out_sb[:, ot, :])

    nc.compile()
    return nc


def make_in_maps(x, gn_gamma, gn_beta, w_qkv, b_qkv, w_proj, b_proj):
    f32 = np.float32
    w_qkv = np.asarray(w_qkv, dtype=f32)
    b_qkv = np.asarray(b_qkv, dtype=f32)
    shared = {
        "w_qkT": np.ascontiguousarray(w_qkv[: 2 * C].T),
        "b_qk": np.ascontiguousarray(b_qkv[: 2 * C].reshape(8, P).T),
        "w_vT": np.ascontiguousarray(
            np.concatenate([w_qkv[2 * C :].T, b_qkv[2 * C :][None, :]], axis=0)
        ),
        "w_pT": np.ascontiguousarray(np.asarray(w_proj, dtype=f32).T),
        "b_p": np.ascontiguousarray(
            np.asarray(b_proj, dtype=f32).reshape(CCH, P).T
        ),
        "gamma": np.ascontiguousarray(
            np.asarray(gn_gamma, dtype=f32).reshape(CCH, P).T
        ),
        "beta": np.ascontiguousarray(
            np.asarray(gn_beta, dtype=f32).reshape(CCH, P).T
        ),
    }
    gsel = np.zeros((P, 8), f32)
    for p in range(P):
        gsel[p, p // GS] = 1.0
    shared["gsel"] = gsel
    shared["gselT"] = np.ascontiguousarray(gsel.T)
    in_maps = []
    for b in range(B):
        m = dict(shared)
        m["x"] = np.ascontiguousarray(np.asarray(x[b], dtype=f32).reshape(C, N))
        in_maps.append(m)
    return in_maps


def kernel(x, gn_gamma, gn_beta, w_qkv, b_qkv, w_proj, b_proj):
    if "nc" not in _CACHE:
        _CACHE["nc"] = build_nc()
    nc = _CACHE["nc"]
    in_maps = make_in_maps(x, gn_gamma, gn_beta, w_qkv, b_qkv, w_proj, b_proj)
    trace = bool(os.environ.get("KERNEL_TRACE"))
    res = run_bass_kernel_spmd(
        nc, in_maps, core_ids=list(range(NCORES)), trace=trace
    )
    _CACHE["last_result"] = res
    out = np.stack([np.asarray(res.results[i]["out"]) for i in range(NCORES)])
    return out.reshape(B, C, 32, 32).astype(np.float32)



# revision 15
# speedup vs baseline: 1.3605x; 1.3605x over previous
"""Trainium2 Bass kernel for nn_AttentionBlock (GroupNorm + attention block),
data-parallel over batch across 8 NeuronCores (one batch element per core).

v3 design notes:
  - S_T head-PAIR packing: heads (2t, 2t+1) occupy PE row groups 0-63 /
    64-127, so their K=64 QK^T matmuls run concurrently (~2x).
  - qk and S interleaved per pair so softmax exp (the ACT/DVE-bound part)
    starts ~18us in and overlaps the whole qkv phase. pT is 4-deep.
  - Softmax exp split across ScalarE (table exp) and VectorE (Schraudolph
    bitcast exp: int16(A*x+B) reinterpreted as bf16, one tensor_scalar).
  - AV lhsT per head = contiguous [v_h | ones] 128 cols; output rows 0-63 =
    ha, rows 64-127 = Z replicated (free-dim bound: the ones cost nothing).
  - Normalization: reciprocal_approx_fast straight off the PSUM Z rows into
    a [P, N] zinv tile (head a -> rows 64-127, head b -> rows 0-63), then
    one fused tensor_mul eviction per head. No gather, no broadcast matmul.
  - k bias dropped (softmax-row invariant); v bias folded into b_proj on
    the host; q bias applied in the PSUM eviction.
  - Weights pre-arranged on host to the SBUF layout so the f32->bf16
    casting DMAs (gpsimd queue) are contiguous; wqk split per pair so
    pair-0 weights land first.
  - PE warm-keeper dummy matmuls with data deps placed to bridge the DMA +
    GroupNorm stats window (keeps HAM at K=8/8).
"""

import os

import numpy as np

import concourse.bass as bass
import concourse.bacc as bacc
import concourse.mybir as mybir
import concourse.tile as tile
from concourse.bass_utils import run_bass_kernel_spmd

F32 = mybir.dt.float32
BF16 = mybir.dt.bfloat16
I16 = mybir.dt.int16
AF = mybir.ActivationFunctionType
ALU = mybir.AluOpType

B = 8
C = 512
N = 1024          # H*W = 32*32
H = 8             # num heads
HD = 64           # head dim
G = 32            # groups
GS = C // G       # channels per group = 16
CCH = 4           # channel chunks of 128
NT = 2            # n tiles of 512
MT = 8            # m tiles of 128
PAIRS = 4
EPS = 1e-5
P = 128
NCORES = 8
SCALE = float(HD) ** -0.5

# Schraudolph bf16 exp: bitcast_bf16(int16(A*(scale*s) + B)) ~ exp(scale*s)
A_EXP = (128.0 / float(np.log(2.0))) * SCALE
B_EXP = 16250.875

EXP_ACT = int(os.environ.get("EXP_ACT", "10"))  # of 16 granules/pair -> ACT
KW1 = int(os.environ.get("KW1", "36"))
KW2 = int(os.environ.get("KW2", "20"))

_CACHE = {}


def build_nc():
    nc = bacc.Bacc(
        "TRN2", target_bir_lowering=False, debug=False, num_devices=NCORES
    )

    x_d = nc.declare_dram_parameter("x", [C, N], F32, isOutput=False)
    # host pre-arranged: [P, CCH, ...] layouts, contiguous DMA targets
    wqk_d = nc.declare_dram_parameter("w_qkr", [P, PAIRS, 2, CCH, P], F32, isOutput=False)
    bq_d = nc.declare_dram_parameter("b_q", [P, CCH], F32, isOutput=False)
    wv_d = nc.declare_dram_parameter("w_vr", [P, CCH, C], F32, isOutput=False)
    wp_d = nc.declare_dram_parameter("w_pr", [P, CCH, C], F32, isOutput=False)
    bp_d = nc.declare_dram_parameter("b_p2", [P, CCH], F32, isOutput=False)
    gm_d = nc.declare_dram_parameter("gamma", [P, CCH], F32, isOutput=False)
    bt_d = nc.declare_dram_parameter("beta", [P, CCH], F32, isOutput=False)
    gsel_d = nc.declare_dram_parameter("gsel", [P, 8], F32, isOutput=False)
    gselT_d = nc.declare_dram_parameter("gselT", [8, P], F32, isOutput=False)
    dums_d = nc.declare_dram_parameter("dums", [2, 512], F32, isOutput=False)
    out_d = nc.declare_dram_parameter("out", [C, N], F32, isOutput=True)
    dbg = bool(os.environ.get("KDBG"))
    if dbg:
        dxn_d = nc.declare_dram_parameter("d_xn", [P, CCH, N], F32, isOutput=True)
        dqk_d = nc.declare_dram_parameter("d_qk", [P, 8, N], F32, isOutput=True)
        dvt_d = nc.declare_dram_parameter("d_vt", [P, MT, 8, P], F32, isOutput=True)
        dpt_d = nc.declare_dram_parameter(
            "d_pt", [P, MT, NT, 2, 512], F32, isOutput=True
        )
        dha_d = nc.declare_dram_parameter("d_ha", [P, CCH, N], F32, isOutput=True)

    with tile.TileContext(nc) as tc:
        with (
            tc.tile_pool(name="singles", bufs=1) as singles,
            tc.tile_pool(name="pa", bufs=3, space="PSUM") as pa,
            tc.tile_pool(name="pav", bufs=2, space="PSUM") as pav,
        ):
            # ---------------- static tiles ----------------
            x_sb = singles.tile([P, CCH, N], F32)
            sq_scr = singles.tile([P, N], F32)
            wqk_sb = singles.tile([P, PAIRS, 2, CCH, P], BF16)
            wv_sb = singles.tile([P, CCH, C], BF16)
            wp_sb = singles.tile([P, CCH, C], BF16)
            bq_sb = singles.tile([P, CCH], F32)
            bp_sb = singles.tile([P, CCH], F32)
            gm_sb = singles.tile([P, CCH], F32)
            bt_sb = singles.tile([P, CCH], F32)
            gsel_sb = singles.tile([P, 8], BF16)
            gselT_sb = singles.tile([8, P], BF16)
            dums = singles.tile([2, 512], BF16)

            xn_sb = singles.tile([P, CCH, N], BF16)
            qk_sb = singles.tile([P, 8, N], BF16)   # slot t<4: q pair t; 4+t: k
            vT_sb = singles.tile([P, MT, 8, P], BF16)  # per head: [v_h | ones]
            pT_t = [
                singles.tile([P, MT, NT, 2, 512], BF16, name=f"pT{i}")
                for i in range(2)
            ]
            zinv_t = [
                singles.tile([64, N], F32, name=f"zinv{i}") for i in range(2)
            ]
            zs_t = [
                singles.tile([64, N], F32, name=f"zs{i}") for i in range(2)
            ]
            ha_sb = singles.tile([P, CCH, N], BF16)
            out_sb = singles.tile([P, CCH, N], F32)

            s12_sb = singles.tile([P, 8], F32)      # cols 0-3 sum, 4-7 sumsq
            s12_bf = singles.tile([P, 8], BF16)
            mu_rs = singles.tile([8, 8], F32)       # cols 0-3 mu, 4-7 rs
            mu_rs_bf = singles.tile([8, 8], BF16)
            tmp8 = singles.tile([8, CCH], F32)
            var_sb = singles.tile([8, CCH], F32)
            lnv_sb = singles.tile([8, CCH], F32)
            s0_sb = singles.tile([P, CCH], F32)
            tmp128 = singles.tile([P, CCH], F32)
            sbias_sb = singles.tile([P, CCH], F32)
            eps_sb = singles.tile([8, 1], F32)

            # ---------------- input DMAs ----------------
            x_v = x_d.ap().rearrange("(cc p) n -> p cc n", p=P)
            for cc in range(CCH):
                nc.sync.dma_start(x_sb[:, cc, :], x_v[:, cc, :])
            nc.gpsimd.dma_start(dums[:], dums_d.ap())
            nc.sync.dma_start(bq_sb[:], bq_d.ap())
            nc.sync.dma_start(bp_sb[:], bp_d.ap())
            nc.sync.dma_start(gm_sb[:], gm_d.ap())
            nc.sync.dma_start(bt_sb[:], bt_d.ap())
            # casting DMAs (f32 dram -> bf16 sbuf) on the gpsimd queue;
            # wqk split per pair so pair 0's q+k weights land first
            for t in range(PAIRS):
                nc.gpsimd.dma_start(wqk_sb[:, t], wqk_d.ap()[:, t])
            nc.gpsimd.dma_start(wv_sb[:], wv_d.ap())
            nc.gpsimd.dma_start(gsel_sb[:], gsel_d.ap())
            nc.gpsimd.dma_start(gselT_sb[:], gselT_d.ap())
            nc.gpsimd.dma_start(wp_sb[:], wp_d.ap())

            nc.vector.memset(eps_sb[:], EPS)
            nc.vector.memset(vT_sb[:, :, :, 64:128], 1.0)

            # PE warm-keeper: dummy matmuls (dums arrives by DMA at t~1us)
            if KW1 > 0:
                ps_w = pa.tile([P, N], F32, tag="ps")
                for _ in range(KW1):
                    nc.tensor.matmul(
                        ps_w[0:P, 0:512], dums[:, 0:P], dums[:], start=True,
                        stop=True,
                    )

            # ---------------- GroupNorm stats ----------------
            for cc in range(CCH):
                nc.vector.reduce_sum(
                    s12_sb[:, cc : cc + 1], x_sb[:, cc, :],
                    axis=mybir.AxisListType.X,
                )
                nc.scalar.activation(
                    sq_scr[:], x_sb[:, cc, :], AF.Square,
                    accum_out=s12_sb[:, 4 + cc : 5 + cc],
                )
            nc.vector.tensor_copy(s12_bf[:], s12_sb[:])
            # warm-keeper #2: depends on s12_bf so it lands in the stats gap
            if KW2 > 0:
                ps_w2 = pa.tile([P, N], F32, tag="ps")
                for _ in range(KW2):
                    nc.tensor.matmul(
                        ps_w2[0:8, 0:512], s12_bf[0:2, :], dums[:], start=True,
                        stop=True,
                    )
            ps_st = pa.tile([P, N], F32, tag="ps")
            nc.tensor.matmul(
                ps_st[0:8, 0:8], gsel_sb[:], s12_bf[:], start=True, stop=True
            )
            inv_cnt = 1.0 / (GS * N)
            # mu = s1/cnt ; var = s2/cnt - mu^2 ; rs = exp(-0.5*ln(var+eps))
            nc.vector.tensor_scalar_mul(mu_rs[:, 0:4], ps_st[0:8, 0:4], inv_cnt)
            nc.vector.tensor_mul(tmp8[:], mu_rs[:, 0:4], mu_rs[:, 0:4])
            nc.vector.scalar_tensor_tensor(
                out=var_sb[:],
                in0=ps_st[0:8, 4:8],
                scalar=inv_cnt,
                in1=tmp8[:],
                op0=ALU.mult,
                op1=ALU.subtract,
            )
            nc.scalar.activation(lnv_sb[:], var_sb[:], AF.Ln, bias=eps_sb[:])
            nc.scalar.activation(mu_rs[:, 4:8], lnv_sb[:], AF.Exp, scale=-0.5)
            nc.vector.tensor_copy(mu_rs_bf[:], mu_rs[:])
            ps_bc = pa.tile([P, N], F32, tag="ps")
            nc.tensor.matmul(
                ps_bc[0:P, 0:8], gselT_sb[:], mu_rs_bf[:], start=True, stop=True
            )
            nc.vector.tensor_mul(s0_sb[:], ps_bc[0:P, 4:8], gm_sb[:])
            nc.vector.tensor_mul(tmp128[:], ps_bc[0:P, 0:4], s0_sb[:])
            nc.vector.tensor_sub(sbias_sb[:], bt_sb[:], tmp128[:])
            for cc in range(CCH):
                nc.vector.tensor_scalar(
                    out=xn_sb[:, cc, :],
                    in0=x_sb[:, cc, :],
                    scalar1=s0_sb[:, cc : cc + 1],
                    scalar2=sbias_sb[:, cc : cc + 1],
                    op0=ALU.mult,
                    op1=ALU.add,
                )

            # ---------------- qk + S, interleaved per pair ----------------
            def qk_group(t, j):
                slot = t if j == 0 else 4 + t
                ps_qk = pa.tile([P, N], F32, tag="ps", name=f"qk{slot}")
                for nt in range(NT):
                    for cc in range(CCH):
                        nc.tensor.matmul(
                            ps_qk[:, nt * 512 : (nt + 1) * 512],
                            wqk_sb[:, t, j, cc, :],
                            xn_sb[:, cc, nt * 512 : (nt + 1) * 512],
                            start=(cc == 0),
                            stop=(cc == CCH - 1),
                        )
                if j == 0:  # q: bias folded into the eviction
                    nc.scalar.activation(
                        qk_sb[:, slot, :], ps_qk[:], AF.Identity,
                        bias=bq_sb[:, t : t + 1],
                    )
                else:       # k: bias dropped (softmax-row invariant)
                    nc.vector.tensor_copy(qk_sb[:, slot, :], ps_qk[:])

            # even ACT/DVE interleave for exp granules: ACT when
            # (g % 8) < ceil(EXP_ACT / 2) over a period-8 pattern
            act_per8 = (EXP_ACT + 1) // 2

            def s_pair(t):
                pT = pT_t[t % 2]
                for mt in range(MT):
                    for nt in range(NT):
                        ps = pa.tile([P, N], F32, tag="ps", name=f"s{t}_{mt}_{nt}")
                        for hh in range(2):
                            po = 64 * hh
                            nc.tensor.matmul(
                                ps[:, hh * 512 : (hh + 1) * 512],
                                qk_sb[po : po + 64, 4 + t, mt * P : (mt + 1) * P],
                                qk_sb[po : po + 64, t, nt * 512 : (nt + 1) * 512],
                                start=True,
                                stop=True,
                            )
                        g = mt * NT + nt
                        dst = pT[:, mt, nt, :, :]
                        if (g % 8) < act_per8:
                            nc.scalar.activation(dst, ps[:], AF.Exp, scale=SCALE)
                        else:
                            nc.vector.tensor_scalar(
                                out=dst.bitcast(I16),
                                in0=ps[:],
                                scalar1=A_EXP,
                                scalar2=B_EXP,
                                op0=ALU.mult,
                                op1=ALU.add,
                            )

            def v_group():
                for mt in range(MT):
                    ps_v = pa.tile([P, N], F32, tag="ps", name=f"v{mt}")
                    for cc in range(CCH):
                        nc.tensor.matmul(
                            ps_v[:, 0:C],
                            xn_sb[:, cc, mt * P : (mt + 1) * P],
                            wv_sb[:, cc, :],
                            start=(cc == 0),
                            stop=(cc == CCH - 1),
                        )
                    nc.scalar.activation(
                        vT_sb[:, mt, :, 0:64], ps_v[:, 0:C], AF.Identity
                    )

            # ---------------- AV + normalize ----------------
            def av_norm(t):
                pT = pT_t[t % 2]
                for hh in range(2):
                    po = 64 * hh
                    zinv = zinv_t[hh]
                    zs = zs_t[hh]
                    for nt in range(NT):
                        ns = slice(nt * 512, (nt + 1) * 512)
                        ps_av = pav.tile(
                            [P, 512], F32, tag="av", name=f"av{t}_{hh}_{nt}"
                        )
                        for mt in range(MT):
                            nc.tensor.matmul(
                                ps_av[:, :],
                                vT_sb[:, mt, 2 * t + hh, :],
                                pT[:, mt, nt, hh, :],
                                start=(mt == 0),
                                stop=(mt == MT - 1),
                            )
                        # stage Z (PSUM rows 64-127) into base-0 SBUF: the
                        # custom-DVE recip only works SBUF->SBUF at base 0,
                        # and tensor_mul in1 must be base-0 (v1 pattern).
                        nc.vector.tensor_copy(zs[:, ns], ps_av[64:128, :])
                        nc.vector.reciprocal_approx_fast(
                            out=zinv[:, ns], in_=zs[:, ns]
                        )
                        nc.vector.tensor_mul(
                            ha_sb[po : po + 64, t, ns],
                            ps_av[0:64, :],
                            zinv[:, ns],
                        )

            # pipeline: S(t) overlaps qk(t) and AV(t-1); pT double-buffered
            qk_group(0, 0)
            qk_group(0, 1)
            s_pair(0)
            qk_group(1, 0)
            qk_group(1, 1)
            s_pair(1)
            v_group()
            av_norm(0)
            qk_group(2, 0)
            qk_group(2, 1)
            s_pair(2)
            av_norm(1)
            qk_group(3, 0)
            qk_group(3, 1)
            s_pair(3)
            av_norm(2)
            av_norm(3)

            if dbg:
                nc.gpsimd.dma_start(dxn_d.ap(), xn_sb[:])
                nc.gpsimd.dma_start(dqk_d.ap(), qk_sb[:])
                nc.gpsimd.dma_start(dvt_d.ap(), vT_sb[:])
                nc.gpsimd.dma_start(dpt_d.ap(), pT_t[0][:])
                nc.gpsimd.dma_start(dha_d.ap(), ha_sb[:])

            # ---------------- proj + bias + residual ----------------
            out_v = out_d.ap().rearrange("(ot p) n -> p ot n", p=P)
            for ot in range(CCH):
                ps_p = pa.tile([P, N], F32, tag="ps", name=f"p{ot}")
                for nt in range(NT):
                    for cc in range(CCH):
                        nc.tensor.matmul(
                            ps_p[:, nt * 512 : (nt + 1) * 512],
                            wp_sb[:, cc, ot * P : (ot + 1) * P],
                            ha_sb[:, cc, nt * 512 : (nt + 1) * 512],
                            start=(cc == 0),
                            stop=(cc == CCH - 1),
                        )
                nc.vector.scalar_tensor_tensor(
                    out=out_sb[:, ot, :],
                    in0=ps_p[:],
                    scalar=bp_sb[:, ot : ot + 1],
                    in1=x_sb[:, ot, :],
                    op0=ALU.add,
                    op1=ALU.add,
                )
                nc.sync.dma_start(out_v[:, ot, :], out_sb[:, ot, :])

    nc.compile()
    return nc


def make_in_maps(x, gn_gamma, gn_beta, w_qkv, b_qkv, w_proj, b_proj):
    f32 = np.float32
    w_qkv = np.asarray(w_qkv, dtype=f32)
    b_qkv = np.asarray(b_qkv, dtype=f32)
    w_proj = np.asarray(w_proj, dtype=f32)
    b_proj = np.asarray(b_proj, dtype=f32)
    b_v = b_qkv[2 * C :]
    bp2 = b_proj + w_proj @ b_v

    def rearr(wT):  # [C(in), O] -> [P, CCH, O] with in-channel = cc*128 + p
        return np.ascontiguousarray(wT.reshape(CCH, P, -1).transpose(1, 0, 2))

    shared = {
        "w_qkr": np.ascontiguousarray(
            w_qkv[: 2 * C].T.reshape(CCH, P, 2, PAIRS, P)
            .transpose(1, 3, 2, 0, 4)
        ),
        "b_q": np.ascontiguousarray(b_qkv[:C].reshape(CCH, P).T),
        "w_vr": rearr(w_qkv[2 * C :].T),
        "w_pr": rearr(w_proj.T),
        "b_p2": np.ascontiguousarray(bp2.reshape(CCH, P).T),
        "gamma": np.ascontiguousarray(
            np.asarray(gn_gamma, dtype=f32).reshape(CCH, P).T
        ),
        "beta": np.ascontiguousarray(
            np.asarray(gn_beta, dtype=f32).reshape(CCH, P).T
        ),
        "dums": np.full((2, 512), 0.5, f32),
    }
    gsel = np.zeros((P, 8), f32)
    for p in range(P):
        gsel[p, p // GS] = 1.0
    shared["gsel"] = gsel
    shared["gselT"] = np.ascontiguousarray(gsel.T)
    in_maps = []
    for b in range(B):
        m = dict(shared)
        m["x"] = np.ascontiguousarray(np.asarray(x[b], dtype=f32).reshape(C, N))
        in_maps.append(m)
    return in_maps


def kernel(x, gn_gamma, gn_beta, w_qkv, b_qkv, w_proj, b_proj):
    if "nc" not in _CACHE:
        _CACHE["nc"] = build_nc()
    nc = _CACHE["nc"]
    in_maps = make_in_maps(x, gn_gamma, gn_beta, w_qkv, b_qkv, w_proj, b_proj)
    trace = bool(os.environ.get("KERNEL_TRACE"))
    res = run_bass_kernel_spmd(
        nc, in_maps, core_ids=list(range(NCORES)), trace=trace
    )
    _CACHE["last_result"] = res
    out = np.stack([np.asarray(res.results[i]["out"]) for i in range(NCORES)])
    return out.reshape(B, C, 32, 32).astype(np.float32)


# revision 17
# speedup vs baseline: 1.5838x; 1.1641x over previous
"""Trainium2 Bass kernel for nn_AttentionBlock (GroupNorm + attention block),
data-parallel over batch across 8 NeuronCores (one batch element per core).

v3 design notes:
  - S_T head-PAIR packing: heads (2t, 2t+1) occupy PE row groups 0-63 /
    64-127, so their K=64 QK^T matmuls run concurrently (~2x).
  - qk and S interleaved per pair so softmax exp (the ACT/DVE-bound part)
    starts ~18us in and overlaps the whole qkv phase. pT is 4-deep.
  - Softmax exp split across ScalarE (table exp) and VectorE (Schraudolph
    bitcast exp: int16(A*x+B) reinterpreted as bf16, one tensor_scalar).
  - AV lhsT per head = contiguous [v_h | ones] 128 cols; output rows 0-63 =
    ha, rows 64-127 = Z replicated (free-dim bound: the ones cost nothing).
  - Normalization: reciprocal_approx_fast straight off the PSUM Z rows into
    a [P, N] zinv tile (head a -> rows 64-127, head b -> rows 0-63), then
    one fused tensor_mul eviction per head. No gather, no broadcast matmul.
  - k bias dropped (softmax-row invariant); v bias folded into b_proj on
    the host; q bias applied in the PSUM eviction.
  - Weights pre-arranged on host to the SBUF layout so the f32->bf16
    casting DMAs (gpsimd queue) are contiguous; wqk split per pair so
    pair-0 weights land first.
  - PE warm-keeper dummy matmuls with data deps placed to bridge the DMA +
    GroupNorm stats window (keeps HAM at K=8/8).
"""

import os

import numpy as np

import concourse.bass as bass
import concourse.bacc as bacc
import concourse.mybir as mybir
import concourse.tile as tile
from concourse.bass_utils import run_bass_kernel_spmd

F32 = mybir.dt.float32
BF16 = mybir.dt.bfloat16
I16 = mybir.dt.int16
AF = mybir.ActivationFunctionType
ALU = mybir.AluOpType

B = 8
C = 512
N = 1024          # H*W = 32*32
H = 8             # num heads
HD = 64           # head dim
G = 32            # groups
GS = C // G       # channels per group = 16
CCH = 4           # channel chunks of 128
NT = 2            # n tiles of 512
MT = 8            # m tiles of 128
PAIRS = 4
EPS = 1e-5
P = 128
NCORES = 8
SCALE = float(HD) ** -0.5

# Schraudolph bf16 exp: bitcast_bf16(int16(A*(scale*s) + B)) ~ exp(scale*s)
A_EXP = (128.0 / float(np.log(2.0))) * SCALE
B_EXP = 16250.875

# per-pair ACT share of the 16 exp granules (rest -> VectorE Schraudolph)
EXP_ACT = [int(v) for v in os.environ.get("EXP_ACT", "8,9,11,13").split(",")]
KW1 = int(os.environ.get("KW1", "40"))
KW2 = int(os.environ.get("KW2", "12"))

_CACHE = {}


def build_nc():
    nc = bacc.Bacc(
        "TRN2", target_bir_lowering=False, debug=False, num_devices=NCORES
    )

    x_d = nc.declare_dram_parameter("x", [C, N], F32, isOutput=False)
    # host pre-arranged: [P, CCH, ...] layouts, contiguous DMA targets
    wqk_d = nc.declare_dram_parameter("w_qkr", [P, PAIRS, 2, CCH, P], F32, isOutput=False)
    bq_d = nc.declare_dram_parameter("b_q", [P, CCH], F32, isOutput=False)
    wv_d = nc.declare_dram_parameter("w_vr", [P, CCH, C], F32, isOutput=False)
    wp_d = nc.declare_dram_parameter("w_pr", [P, CCH, C], F32, isOutput=False)
    bp_d = nc.declare_dram_parameter("b_p2", [P, CCH], F32, isOutput=False)
    gm_d = nc.declare_dram_parameter("gamma", [P, CCH], F32, isOutput=False)
    bt_d = nc.declare_dram_parameter("beta", [P, CCH], F32, isOutput=False)
    gsel_d = nc.declare_dram_parameter("gsel", [P, 8], F32, isOutput=False)
    gselT_d = nc.declare_dram_parameter("gselT", [8, P], F32, isOutput=False)
    dums_d = nc.declare_dram_parameter("dums", [2, 512], BF16, isOutput=False)
    out_d = nc.declare_dram_parameter("out", [C, N], F32, isOutput=True)
    dbg = bool(os.environ.get("KDBG"))
    if dbg:
        dxn_d = nc.declare_dram_parameter("d_xn", [P, CCH, N], F32, isOutput=True)
        dqk_d = nc.declare_dram_parameter("d_qk", [P, 8, N], F32, isOutput=True)
        dvt_d = nc.declare_dram_parameter("d_vt", [P, MT, 8, P], F32, isOutput=True)
        dpt_d = nc.declare_dram_parameter(
            "d_pt", [P, MT, NT, 2, 512], F32, isOutput=True
        )
        dha_d = nc.declare_dram_parameter("d_ha", [P, CCH, N], F32, isOutput=True)

    with tile.TileContext(nc) as tc:
        with (
            tc.tile_pool(name="singles", bufs=1) as singles,
            tc.tile_pool(name="pa", bufs=3, space="PSUM") as pa,
            tc.tile_pool(name="pav", bufs=2, space="PSUM") as pav,
        ):
            # ---------------- static tiles ----------------
            x_sb = singles.tile([P, CCH, N], F32)
            sq_scr = singles.tile([P, N], F32)
            wqk_sb = singles.tile([P, PAIRS, 2, CCH, P], BF16)
            wv_sb = singles.tile([P, CCH, C], BF16)
            wp_sb = singles.tile([P, CCH, C], BF16)
            bq_sb = singles.tile([P, CCH], F32)
            bp_sb = singles.tile([P, CCH], F32)
            gm_sb = singles.tile([P, CCH], F32)
            bt_sb = singles.tile([P, CCH], F32)
            gsel_sb = singles.tile([P, 8], BF16)
            gselT_sb = singles.tile([8, P], BF16)
            dums = singles.tile([2, 512], BF16)

            xn_sb = singles.tile([P, CCH, N], BF16)
            qk_sb = singles.tile([P, 8, N], BF16)   # slot t<4: q pair t; 4+t: k
            vT_sb = singles.tile([P, MT, 8, P], BF16)  # per head: [v_h | ones]
            pT_t = [
                singles.tile([P, MT, NT, 2, 512], BF16, name=f"pT{i}")
                for i in range(2)
            ]
            zinv_t = [
                singles.tile([64, N], F32, name=f"zinv{i}") for i in range(2)
            ]
            zs_t = [
                singles.tile([64, N], F32, name=f"zs{i}") for i in range(2)
            ]
            ha_sb = singles.tile([P, CCH, N], BF16)
            out_sb = singles.tile([P, CCH, N], F32)

            s12_sb = singles.tile([P, 8], F32)      # cols 0-3 sum, 4-7 sumsq
            s12_bf = singles.tile([P, 8], BF16)
            mu_rs = singles.tile([8, 8], F32)       # cols 0-3 mu, 4-7 rs
            mu_rs_bf = singles.tile([8, 8], BF16)
            tmp8 = singles.tile([8, CCH], F32)
            var_sb = singles.tile([8, CCH], F32)
            lnv_sb = singles.tile([8, CCH], F32)
            s0_sb = singles.tile([P, CCH], F32)
            tmp128 = singles.tile([P, CCH], F32)
            sbias_sb = singles.tile([P, CCH], F32)
            eps_sb = singles.tile([8, 1], F32)

            # ---------------- input DMAs ----------------
            nc.sync.dma_start(dums[:], dums_d.ap())
            x_v = x_d.ap().rearrange("(cc p) n -> p cc n", p=P)
            for cc in range(CCH):
                nc.sync.dma_start(x_sb[:, cc, :], x_v[:, cc, :])
            nc.sync.dma_start(bq_sb[:], bq_d.ap())
            nc.sync.dma_start(bp_sb[:], bp_d.ap())
            nc.sync.dma_start(gm_sb[:], gm_d.ap())
            nc.sync.dma_start(bt_sb[:], bt_d.ap())
            # casting DMAs (f32 dram -> bf16 sbuf) on the gpsimd queue;
            # wqk split per pair so pair 0's q+k weights land first
            for t in range(PAIRS):
                nc.gpsimd.dma_start(wqk_sb[:, t], wqk_d.ap()[:, t])
            nc.gpsimd.dma_start(wv_sb[:], wv_d.ap())
            nc.gpsimd.dma_start(gsel_sb[:], gsel_d.ap())
            nc.gpsimd.dma_start(gselT_sb[:], gselT_d.ap())
            nc.gpsimd.dma_start(wp_sb[:], wp_d.ap())

            nc.vector.memset(eps_sb[:], EPS)
            nc.vector.memset(vT_sb[:, :, :, 64:128], 1.0)

            # PE warm-keeper: dummy matmuls (dums arrives by DMA at t~1us)
            if KW1 > 0:
                ps_w = pa.tile([P, N], F32, tag="ps")
                for _ in range(KW1):
                    nc.tensor.matmul(
                        ps_w[0:P, 0:512], dums[:, 0:P], dums[:], start=True,
                        stop=True,
                    )

            # ---------------- GroupNorm stats ----------------
            for cc in range(CCH):
                nc.vector.reduce_sum(
                    s12_sb[:, cc : cc + 1], x_sb[:, cc, :],
                    axis=mybir.AxisListType.X,
                )
                nc.scalar.activation(
                    sq_scr[:], x_sb[:, cc, :], AF.Square,
                    accum_out=s12_sb[:, 4 + cc : 5 + cc],
                )
            nc.vector.tensor_copy(s12_bf[:], s12_sb[:])
            # warm-keeper #2: depends on s12_bf so it lands in the stats gap
            if KW2 > 0:
                ps_w2 = pa.tile([P, N], F32, tag="ps")
                for _ in range(KW2):
                    nc.tensor.matmul(
                        ps_w2[0:8, 0:512], s12_bf[0:2, :], dums[:], start=True,
                        stop=True,
                    )
            ps_st = pa.tile([P, N], F32, tag="ps")
            nc.tensor.matmul(
                ps_st[0:8, 0:8], gsel_sb[:], s12_bf[:], start=True, stop=True
            )
            inv_cnt = 1.0 / (GS * N)
            # mu = s1/cnt ; var = s2/cnt - mu^2 ; rs = exp(-0.5*ln(var+eps))
            nc.vector.tensor_scalar_mul(mu_rs[:, 0:4], ps_st[0:8, 0:4], inv_cnt)
            nc.vector.tensor_mul(tmp8[:], mu_rs[:, 0:4], mu_rs[:, 0:4])
            nc.vector.scalar_tensor_tensor(
                out=var_sb[:],
                in0=ps_st[0:8, 4:8],
                scalar=inv_cnt,
                in1=tmp8[:],
                op0=ALU.mult,
                op1=ALU.subtract,
            )
            nc.scalar.activation(lnv_sb[:], var_sb[:], AF.Ln, bias=eps_sb[:])
            nc.scalar.activation(mu_rs[:, 4:8], lnv_sb[:], AF.Exp, scale=-0.5)
            nc.vector.tensor_copy(mu_rs_bf[:], mu_rs[:])
            ps_bc = pa.tile([P, N], F32, tag="ps")
            nc.tensor.matmul(
                ps_bc[0:P, 0:8], gselT_sb[:], mu_rs_bf[:], start=True, stop=True
            )
            nc.vector.tensor_mul(s0_sb[:], ps_bc[0:P, 4:8], gm_sb[:])
            nc.vector.tensor_mul(tmp128[:], ps_bc[0:P, 0:4], s0_sb[:])
            nc.vector.tensor_sub(sbias_sb[:], bt_sb[:], tmp128[:])
            for cc in range(CCH):
                nc.vector.tensor_scalar(
                    out=xn_sb[:, cc, :],
                    in0=x_sb[:, cc, :],
                    scalar1=s0_sb[:, cc : cc + 1],
                    scalar2=sbias_sb[:, cc : cc + 1],
                    op0=ALU.mult,
                    op1=ALU.add,
                )

            # ---------------- qk + S, interleaved per pair ----------------
            def qk_group(t, j):
                slot = t if j == 0 else 4 + t
                ps_qk = pa.tile([P, N], F32, tag="ps", name=f"qk{slot}")
                for nt in range(NT):
                    for cc in range(CCH):
                        nc.tensor.matmul(
                            ps_qk[:, nt * 512 : (nt + 1) * 512],
                            wqk_sb[:, t, j, cc, :],
                            xn_sb[:, cc, nt * 512 : (nt + 1) * 512],
                            start=(cc == 0),
                            stop=(cc == CCH - 1),
                        )
                if j == 0:  # q: bias folded into the eviction
                    nc.scalar.activation(
                        qk_sb[:, slot, :], ps_qk[:], AF.Identity,
                        bias=bq_sb[:, t : t + 1],
                    )
                else:       # k: bias dropped (softmax-row invariant)
                    nc.scalar.activation(qk_sb[:, slot, :], ps_qk[:], AF.Identity)

            def s_pair(t):
                act_of16 = EXP_ACT[t % len(EXP_ACT)]
                pT = pT_t[t % 2]
                for mt in range(MT):
                    for nt in range(NT):
                        ps = pa.tile([P, N], F32, tag="ps", name=f"s{t}_{mt}_{nt}")
                        for hh in range(2):
                            po = 64 * hh
                            nc.tensor.matmul(
                                ps[:, hh * 512 : (hh + 1) * 512],
                                qk_sb[po : po + 64, 4 + t, mt * P : (mt + 1) * P],
                                qk_sb[po : po + 64, t, nt * 512 : (nt + 1) * 512],
                                start=True,
                                stop=True,
                            )
                        g = mt * NT + nt
                        dst = pT[:, mt, nt, :, :]
                        if (g * act_of16) % 16 < act_of16:
                            nc.scalar.activation(dst, ps[:], AF.Exp, scale=SCALE)
                        else:
                            nc.vector.tensor_scalar(
                                out=dst.bitcast(I16),
                                in0=ps[:],
                                scalar1=A_EXP,
                                scalar2=B_EXP,
                                op0=ALU.mult,
                                op1=ALU.add,
                            )

            def v_group():
                for mt in range(MT):
                    ps_v = pa.tile([P, N], F32, tag="ps", name=f"v{mt}")
                    for cc in range(CCH):
                        nc.tensor.matmul(
                            ps_v[:, 0:C],
                            xn_sb[:, cc, mt * P : (mt + 1) * P],
                            wv_sb[:, cc, :],
                            start=(cc == 0),
                            stop=(cc == CCH - 1),
                        )
                    nc.scalar.activation(
                        vT_sb[:, mt, :, 0:64], ps_v[:, 0:C], AF.Identity
                    )

            # ---------------- AV + normalize ----------------
            def av_norm(t):
                pT = pT_t[t % 2]
                for hh in range(2):
                    po = 64 * hh
                    zinv = zinv_t[hh]
                    zs = zs_t[hh]
                    for nt in range(NT):
                        ns = slice(nt * 512, (nt + 1) * 512)
                        ps_av = pav.tile(
                            [P, 512], F32, tag="av", name=f"av{t}_{hh}_{nt}"
                        )
                        for mt in range(MT):
                            nc.tensor.matmul(
                                ps_av[:, :],
                                vT_sb[:, mt, 2 * t + hh, :],
                                pT[:, mt, nt, hh, :],
                                start=(mt == 0),
                                stop=(mt == MT - 1),
                            )
                        # stage Z (PSUM rows 64-127) into base-0 SBUF: the
                        # custom-DVE recip only works SBUF->SBUF at base 0,
                        # and tensor_mul in1 must be base-0 (v1 pattern).
                        nc.vector.tensor_copy(zs[:, ns], ps_av[64:128, :])
                        nc.vector.reciprocal_approx_fast(
                            out=zinv[:, ns], in_=zs[:, ns]
                        )
                        nc.vector.tensor_mul(
                            ha_sb[po : po + 64, t, ns],
                            ps_av[0:64, :],
                            zinv[:, ns],
                        )

            # pipeline: S(t) overlaps qk(t) and AV(t-1); pT double-buffered
            qk_group(0, 0)
            qk_group(0, 1)
            s_pair(0)
            qk_group(1, 0)
            qk_group(1, 1)
            s_pair(1)
            v_group()
            av_norm(0)
            qk_group(2, 0)
            qk_group(2, 1)
            s_pair(2)
            av_norm(1)
            qk_group(3, 0)
            qk_group(3, 1)
            s_pair(3)
            av_norm(2)
            av_norm(3)

            if dbg:
                nc.gpsimd.dma_start(dxn_d.ap(), xn_sb[:])
                nc.gpsimd.dma_start(dqk_d.ap(), qk_sb[:])
                nc.gpsimd.dma_start(dvt_d.ap(), vT_sb[:])
                nc.gpsimd.dma_start(dpt_d.ap(), pT_t[0][:])
                nc.gpsimd.dma_start(dha_d.ap(), ha_sb[:])

            # ---------------- proj + bias + residual ----------------
            out_v = out_d.ap().rearrange("(ot p) n -> p ot n", p=P)
            for ot in range(CCH):
                ps_p = pa.tile([P, N], F32, tag="ps", name=f"p{ot}")
                for nt in range(NT):
                    for cc in range(CCH):
                        nc.tensor.matmul(
                            ps_p[:, nt * 512 : (nt + 1) * 512],
                            wp_sb[:, cc, ot * P : (ot + 1) * P],
                            ha_sb[:, cc, nt * 512 : (nt + 1) * 512],
                            start=(cc == 0),
                            stop=(cc == CCH - 1),
                        )
                nc.vector.scalar_tensor_tensor(
                    out=out_sb[:, ot, :],
                    in0=ps_p[:],
                    scalar=bp_sb[:, ot : ot + 1],
                    in1=x_sb[:, ot, :],
                    op0=ALU.add,
                    op1=ALU.add,
                )
                nc.sync.dma_start(out_v[:, ot, :], out_sb[:, ot, :])

    nc.compile()
    return nc


def make_in_maps(x, gn_gamma, gn_beta, w_qkv, b_qkv, w_proj, b_proj):
    f32 = np.float32
    w_qkv = np.asarray(w_qkv, dtype=f32)
    b_qkv = np.asarray(b_qkv, dtype=f32)
    w_proj = np.asarray(w_proj, dtype=f32)
    b_proj = np.asarray(b_proj, dtype=f32)
    b_v = b_qkv[2 * C :]
    bp2 = b_proj + w_proj @ b_v

    def rearr(wT):  # [C(in), O] -> [P, CCH, O] with in-channel = cc*128 + p
        return np.ascontiguousarray(wT.reshape(CCH, P, -1).transpose(1, 0, 2))

    shared = {
        "w_qkr": np.ascontiguousarray(
            w_qkv[: 2 * C].T.reshape(CCH, P, 2, PAIRS, P)
            .transpose(1, 3, 2, 0, 4)
        ),
        "b_q": np.ascontiguousarray(b_qkv[:C].reshape(CCH, P).T),
        "w_vr": rearr(w_qkv[2 * C :].T),
        "w_pr": rearr(w_proj.T),
        "b_p2": np.ascontiguousarray(bp2.reshape(CCH, P).T),
        "gamma": np.ascontiguousarray(
            np.asarray(gn_gamma, dtype=f32).reshape(CCH, P).T
        ),
        "beta": np.ascontiguousarray(
            np.asarray(gn_beta, dtype=f32).reshape(CCH, P).T
        ),
        "dums": np.full((2, 512), 0.5, __import__("ml_dtypes").bfloat16),
    }
    gsel = np.zeros((P, 8), f32)
    for p in range(P):
        gsel[p, p // GS] = 1.0
    shared["gsel"] = gsel
    shared["gselT"] = np.ascontiguousarray(gsel.T)
    in_maps = []
    for b in range(B):
        m = dict(shared)
        m["x"] = np.ascontiguousarray(np.asarray(x[b], dtype=f32).reshape(C, N))
        in_maps.append(m)
    return in_maps


def kernel(x, gn_gamma, gn_beta, w_qkv, b_qkv, w_proj, b_proj):
    if "nc" not in _CACHE:
        _CACHE["nc"] = build_nc()
    nc = _CACHE["nc"]
    in_maps = make_in_maps(x, gn_gamma, gn_beta, w_qkv, b_qkv, w_proj, b_proj)
    trace = bool(os.environ.get("KERNEL_TRACE"))
    res = run_bass_kernel_spmd(
        nc, in_maps, core_ids=list(range(NCORES)), trace=trace
    )
    _CACHE["last_result"] = res
    out = np.stack([np.asarray(res.results[i]["out"]) for i in range(NCORES)])
    return out.reshape(B, C, 32, 32).astype(np.float32)


# revision 18
# speedup vs baseline: 1.6323x; 1.0306x over previous
"""Trainium2 Bass kernel for nn_AttentionBlock (GroupNorm + attention block),
data-parallel over batch across 8 NeuronCores (one batch element per core).

v3 design notes:
  - S_T head-PAIR packing: heads (2t, 2t+1) occupy PE row groups 0-63 /
    64-127, so their K=64 QK^T matmuls run concurrently (~2x).
  - qk and S interleaved per pair so softmax exp (the ACT/DVE-bound part)
    starts ~18us in and overlaps the whole qkv phase. pT is 4-deep.
  - Softmax exp split across ScalarE (table exp) and VectorE (Schraudolph
    bitcast exp: int16(A*x+B) reinterpreted as bf16, one tensor_scalar).
  - AV lhsT per head = contiguous [v_h | ones] 128 cols; output rows 0-63 =
    ha, rows 64-127 = Z replicated (free-dim bound: the ones cost nothing).
  - Normalization: reciprocal_approx_fast straight off the PSUM Z rows into
    a [P, N] zinv tile (head a -> rows 64-127, head b -> rows 0-63), then
    one fused tensor_mul eviction per head. No gather, no broadcast matmul.
  - k bias dropped (softmax-row invariant); v bias folded into b_proj on
    the host; q bias applied in the PSUM eviction.
  - Weights pre-arranged on host to the SBUF layout so the f32->bf16
    casting DMAs (gpsimd queue) are contiguous; wqk split per pair so
    pair-0 weights land first.
  - PE warm-keeper dummy matmuls with data deps placed to bridge the DMA +
    GroupNorm stats window (keeps HAM at K=8/8).
"""

import os

import numpy as np

import concourse.bass as bass
import concourse.bacc as bacc
import concourse.mybir as mybir
import concourse.tile as tile
from concourse.bass_utils import run_bass_kernel_spmd

F32 = mybir.dt.float32
BF16 = mybir.dt.bfloat16
I16 = mybir.dt.int16
AF = mybir.ActivationFunctionType
ALU = mybir.AluOpType

B = 8
C = 512
N = 1024          # H*W = 32*32
H = 8             # num heads
HD = 64           # head dim
G = 32            # groups
GS = C // G       # channels per group = 16
CCH = 4           # channel chunks of 128
NT = 2            # n tiles of 512
MT = 8            # m tiles of 128
PAIRS = 4
EPS = 1e-5
P = 128
NCORES = 8
SCALE = float(HD) ** -0.5

# Schraudolph bf16 exp: bitcast_bf16(int16(A*(scale*s) + B)) ~ exp(scale*s)
A_EXP = (128.0 / float(np.log(2.0))) * SCALE
B_EXP = 16250.875

# per-pair ACT share of the 16 exp granules (rest -> VectorE Schraudolph)
EXP_ACT = [int(v) for v in os.environ.get("EXP_ACT", "8,9,11,13").split(",")]
KW1 = int(os.environ.get("KW1", "45"))
KW2 = int(os.environ.get("KW2", "8"))

_CACHE = {}


def build_nc():
    nc = bacc.Bacc(
        "TRN2", target_bir_lowering=False, debug=False, num_devices=NCORES
    )

    x_d = nc.declare_dram_parameter("x", [C, N], F32, isOutput=False)
    # host pre-arranged: [P, CCH, ...] layouts, contiguous DMA targets
    wqk_d = nc.declare_dram_parameter("w_qkr", [P, PAIRS, 2, CCH, P], F32, isOutput=False)
    bq_d = nc.declare_dram_parameter("b_q", [P, CCH], F32, isOutput=False)
    wv_d = nc.declare_dram_parameter("w_vr", [P, CCH, C], F32, isOutput=False)
    wp_d = nc.declare_dram_parameter("w_pr", [P, CCH, C], F32, isOutput=False)
    bp_d = nc.declare_dram_parameter("b_p2", [P, CCH], F32, isOutput=False)
    gm_d = nc.declare_dram_parameter("gamma", [P, CCH], F32, isOutput=False)
    bt_d = nc.declare_dram_parameter("beta", [P, CCH], F32, isOutput=False)
    gsel_d = nc.declare_dram_parameter("gsel", [P, 8], F32, isOutput=False)
    gselT_d = nc.declare_dram_parameter("gselT", [8, P], F32, isOutput=False)
    dums_d = nc.declare_dram_parameter("dums", [P, 512], BF16, isOutput=False)
    out_d = nc.declare_dram_parameter("out", [C, N], F32, isOutput=True)
    dbg = bool(os.environ.get("KDBG"))
    if dbg:
        dxn_d = nc.declare_dram_parameter("d_xn", [P, CCH, N], F32, isOutput=True)
        dqk_d = nc.declare_dram_parameter("d_qk", [P, 8, N], F32, isOutput=True)
        dvt_d = nc.declare_dram_parameter("d_vt", [P, MT, 8, P], F32, isOutput=True)
        dpt_d = nc.declare_dram_parameter(
            "d_pt", [P, MT, NT, 2, 512], F32, isOutput=True
        )
        dha_d = nc.declare_dram_parameter("d_ha", [P, CCH, N], F32, isOutput=True)

    with tile.TileContext(nc) as tc:
        with (
            tc.tile_pool(name="singles", bufs=1) as singles,
            tc.tile_pool(name="pa", bufs=3, space="PSUM") as pa,
            tc.tile_pool(name="pav", bufs=2, space="PSUM") as pav,
        ):
            # ---------------- static tiles ----------------
            x_sb = singles.tile([P, CCH, N], F32)
            sq_scr = singles.tile([P, N], F32)
            wqk_sb = singles.tile([P, PAIRS, 2, CCH, P], BF16)
            wv_sb = singles.tile([P, CCH, C], BF16)
            wp_sb = singles.tile([P, CCH, C], BF16)
            bq_sb = singles.tile([P, CCH], F32)
            bp_sb = singles.tile([P, CCH], F32)
            gm_sb = singles.tile([P, CCH], F32)
            bt_sb = singles.tile([P, CCH], F32)
            gsel_sb = singles.tile([P, 8], BF16)
            gselT_sb = singles.tile([8, P], BF16)
            dums = singles.tile([P, 512], BF16)

            xn_sb = singles.tile([P, CCH, N], BF16)
            qk_sb = singles.tile([P, 8, N], BF16)   # slot t<4: q pair t; 4+t: k
            vT_sb = singles.tile([P, MT, 8, P], BF16)  # per head: [v_h | ones]
            pT_t = [
                singles.tile([P, MT, NT, 2, 512], BF16, name=f"pT{i}")
                for i in range(2)
            ]
            zinv_t = [
                singles.tile([64, N], F32, name=f"zinv{i}") for i in range(2)
            ]
            zs_t = [
                singles.tile([64, N], F32, name=f"zs{i}") for i in range(2)
            ]
            ha_sb = singles.tile([P, CCH, N], BF16)
            out_sb = singles.tile([P, CCH, N], F32)

            s12_sb = singles.tile([P, 8], F32)      # cols 0-3 sum, 4-7 sumsq
            s12_bf = singles.tile([P, 8], BF16)
            mu_rs = singles.tile([8, 8], F32)       # cols 0-3 mu, 4-7 rs
            mu_rs_bf = singles.tile([8, 8], BF16)
            tmp8 = singles.tile([8, CCH], F32)
            var_sb = singles.tile([8, CCH], F32)
            lnv_sb = singles.tile([8, CCH], F32)
            s0_sb = singles.tile([P, CCH], F32)
            tmp128 = singles.tile([P, CCH], F32)
            sbias_sb = singles.tile([P, CCH], F32)
            eps_sb = singles.tile([8, 1], F32)

            # ---------------- input DMAs ----------------
            nc.sync.dma_start(dums[:], dums_d.ap())
            x_v = x_d.ap().rearrange("(cc p) n -> p cc n", p=P)
            for cc in range(CCH):
                nc.sync.dma_start(x_sb[:, cc, :], x_v[:, cc, :])
            nc.sync.dma_start(bq_sb[:], bq_d.ap())
            nc.sync.dma_start(bp_sb[:], bp_d.ap())
            nc.sync.dma_start(gm_sb[:], gm_d.ap())
            nc.sync.dma_start(bt_sb[:], bt_d.ap())
            # casting DMAs (f32 dram -> bf16 sbuf) on the gpsimd queue;
            # wqk split per pair so pair 0's q+k weights land first
            for t in range(PAIRS):
                nc.gpsimd.dma_start(wqk_sb[:, t], wqk_d.ap()[:, t])
            nc.gpsimd.dma_start(wv_sb[:], wv_d.ap())
            nc.gpsimd.dma_start(gsel_sb[:], gsel_d.ap())
            nc.gpsimd.dma_start(gselT_sb[:], gselT_d.ap())
            nc.gpsimd.dma_start(wp_sb[:], wp_d.ap())

            nc.vector.memset(eps_sb[:], EPS)
            nc.vector.memset(vT_sb[:, :, :, 64:128], 1.0)

            # PE warm-keeper: dummy matmuls (dums arrives by DMA at t~1us)
            if KW1 > 0:
                ps_w = pa.tile([P, N], F32, tag="ps")
                for _ in range(KW1):
                    nc.tensor.matmul(
                        ps_w[0:P, 0:512], dums[:, 0:P], dums[:], start=True,
                        stop=True,
                    )

            # ---------------- GroupNorm stats ----------------
            for cc in range(CCH):
                nc.vector.reduce_sum(
                    s12_sb[:, cc : cc + 1], x_sb[:, cc, :],
                    axis=mybir.AxisListType.X,
                )
                nc.scalar.activation(
                    sq_scr[:], x_sb[:, cc, :], AF.Square,
                    accum_out=s12_sb[:, 4 + cc : 5 + cc],
                )
            nc.vector.tensor_copy(s12_bf[:], s12_sb[:])
            # warm-keeper #2: depends on s12_bf so it lands in the stats gap
            if KW2 > 0:
                ps_w2 = pa.tile([P, N], F32, tag="ps")
                for _ in range(KW2):
                    nc.tensor.matmul(
                        ps_w2[0:8, 0:512], s12_bf[0:2, :], dums[0:2, :],
                        start=True, stop=True,
                    )
            ps_st = pa.tile([P, N], F32, tag="ps")
            nc.tensor.matmul(
                ps_st[0:8, 0:8], gsel_sb[:], s12_bf[:], start=True, stop=True
            )
            inv_cnt = 1.0 / (GS * N)
            # mu = s1/cnt ; var = s2/cnt - mu^2 ; rs = exp(-0.5*ln(var+eps))
            nc.vector.tensor_scalar_mul(mu_rs[:, 0:4], ps_st[0:8, 0:4], inv_cnt)
            nc.vector.tensor_mul(tmp8[:], mu_rs[:, 0:4], mu_rs[:, 0:4])
            nc.vector.scalar_tensor_tensor(
                out=var_sb[:],
                in0=ps_st[0:8, 4:8],
                scalar=inv_cnt,
                in1=tmp8[:],
                op0=ALU.mult,
                op1=ALU.subtract,
            )
            nc.scalar.activation(lnv_sb[:], var_sb[:], AF.Ln, bias=eps_sb[:])
            nc.scalar.activation(mu_rs[:, 4:8], lnv_sb[:], AF.Exp, scale=-0.5)
            nc.vector.tensor_copy(mu_rs_bf[:], mu_rs[:])
            ps_bc = pa.tile([P, N], F32, tag="ps")
            nc.tensor.matmul(
                ps_bc[0:P, 0:8], gselT_sb[:], mu_rs_bf[:], start=True, stop=True
            )
            nc.vector.tensor_mul(s0_sb[:], ps_bc[0:P, 4:8], gm_sb[:])
            nc.vector.tensor_mul(tmp128[:], ps_bc[0:P, 0:4], s0_sb[:])
            nc.vector.tensor_sub(sbias_sb[:], bt_sb[:], tmp128[:])
            for cc in range(CCH):
                nc.vector.tensor_scalar(
                    out=xn_sb[:, cc, :],
                    in0=x_sb[:, cc, :],
                    scalar1=s0_sb[:, cc : cc + 1],
                    scalar2=sbias_sb[:, cc : cc + 1],
                    op0=ALU.mult,
                    op1=ALU.add,
                )

            # ---------------- qk + S, interleaved per pair ----------------
            def qk_group(t, j):
                slot = t if j == 0 else 4 + t
                ps_qk = pa.tile([P, N], F32, tag="ps", name=f"qk{slot}")
                for nt in range(NT):
                    for cc in range(CCH):
                        nc.tensor.matmul(
                            ps_qk[:, nt * 512 : (nt + 1) * 512],
                            wqk_sb[:, t, j, cc, :],
                            xn_sb[:, cc, nt * 512 : (nt + 1) * 512],
                            start=(cc == 0),
                            stop=(cc == CCH - 1),
                        )
                if j == 0:  # q: bias folded into the eviction
                    nc.scalar.activation(
                        qk_sb[:, slot, :], ps_qk[:], AF.Identity,
                        bias=bq_sb[:, t : t + 1],
                    )
                else:       # k: bias dropped (softmax-row invariant)
                    nc.scalar.activation(qk_sb[:, slot, :], ps_qk[:], AF.Identity)

            def s_pair(t):
                act_of16 = EXP_ACT[t % len(EXP_ACT)]
                pT = pT_t[t % 2]
                for mt in range(MT):
                    for nt in range(NT):
                        ps = pa.tile([P, N], F32, tag="ps", name=f"s{t}_{mt}_{nt}")
                        for hh in range(2):
                            po = 64 * hh
                            nc.tensor.matmul(
                                ps[:, hh * 512 : (hh + 1) * 512],
                                qk_sb[po : po + 64, 4 + t, mt * P : (mt + 1) * P],
                                qk_sb[po : po + 64, t, nt * 512 : (nt + 1) * 512],
                                start=True,
                                stop=True,
                            )
                        g = mt * NT + nt
                        dst = pT[:, mt, nt, :, :]
                        if (g * act_of16) % 16 < act_of16:
                            nc.scalar.activation(dst, ps[:], AF.Exp, scale=SCALE)
                        else:
                            nc.vector.tensor_scalar(
                                out=dst.bitcast(I16),
                                in0=ps[:],
                                scalar1=A_EXP,
                                scalar2=B_EXP,
                                op0=ALU.mult,
                                op1=ALU.add,
                            )

            def v_group():
                for mt in range(MT):
                    ps_v = pa.tile([P, N], F32, tag="ps", name=f"v{mt}")
                    for cc in range(CCH):
                        nc.tensor.matmul(
                            ps_v[:, 0:C],
                            xn_sb[:, cc, mt * P : (mt + 1) * P],
                            wv_sb[:, cc, :],
                            start=(cc == 0),
                            stop=(cc == CCH - 1),
                        )
                    nc.scalar.activation(
                        vT_sb[:, mt, :, 0:64], ps_v[:, 0:C], AF.Identity
                    )

            # ---------------- AV + normalize ----------------
            def av_norm(t):
                pT = pT_t[t % 2]
                for hh in range(2):
                    po = 64 * hh
                    zinv = zinv_t[hh]
                    zs = zs_t[hh]
                    for nt in range(NT):
                        ns = slice(nt * 512, (nt + 1) * 512)
                        ps_av = pav.tile(
                            [P, 512], F32, tag="av", name=f"av{t}_{hh}_{nt}"
                        )
                        for mt in range(MT):
                            nc.tensor.matmul(
                                ps_av[:, :],
                                vT_sb[:, mt, 2 * t + hh, :],
                                pT[:, mt, nt, hh, :],
                                start=(mt == 0),
                                stop=(mt == MT - 1),
                            )
                        # stage Z (PSUM rows 64-127) into base-0 SBUF: the
                        # custom-DVE recip only works SBUF->SBUF at base 0,
                        # and tensor_mul in1 must be base-0 (v1 pattern).
                        nc.vector.tensor_copy(zs[:, ns], ps_av[64:128, :])
                        nc.vector.reciprocal_approx_fast(
                            out=zinv[:, ns], in_=zs[:, ns]
                        )
                        nc.vector.tensor_mul(
                            ha_sb[po : po + 64, t, ns],
                            ps_av[0:64, :],
                            zinv[:, ns],
                        )

            # pipeline: S(t) overlaps qk(t) and AV(t-1); pT double-buffered
            qk_group(0, 0)
            qk_group(0, 1)
            s_pair(0)
            qk_group(1, 0)
            qk_group(1, 1)
            s_pair(1)
            v_group()
            av_norm(0)
            qk_group(2, 0)
            qk_group(2, 1)
            s_pair(2)
            av_norm(1)
            qk_group(3, 0)
            qk_group(3, 1)
            s_pair(3)
            av_norm(2)
            av_norm(3)

            if dbg:
                nc.gpsimd.dma_start(dxn_d.ap(), xn_sb[:])
                nc.gpsimd.dma_start(dqk_d.ap(), qk_sb[:])
                nc.gpsimd.dma_start(dvt_d.ap(), vT_sb[:])
                nc.gpsimd.dma_start(dpt_d.ap(), pT_t[0][:])
                nc.gpsimd.dma_start(dha_d.ap(), ha_sb[:])

            # ---------------- proj + bias + residual ----------------
            out_v = out_d.ap().rearrange("(ot p) n -> p ot n", p=P)
            for ot in range(CCH):
                ps_p = pa.tile([P, N], F32, tag="ps", name=f"p{ot}")
                for nt in range(NT):
                    for cc in range(CCH):
                        nc.tensor.matmul(
                            ps_p[:, nt * 512 : (nt + 1) * 512],
                            wp_sb[:, cc, ot * P : (ot + 1) * P],
                            ha_sb[:, cc, nt * 512 : (nt + 1) * 512],
                            start=(cc == 0),
                            stop=(cc == CCH - 1),
                        )
                nc.vector.scalar_tensor_tensor(
                    out=out_sb[:, ot, :],
                    in0=ps_p[:],
                    scalar=bp_sb[:, ot : ot + 1],
                    in1=x_sb[:, ot, :],
                    op0=ALU.add,
                    op1=ALU.add,
                )
                nc.sync.dma_start(out_v[:, ot, :], out_sb[:, ot, :])

    nc.compile()
    return nc


def make_in_maps(x, gn_gamma, gn_beta, w_qkv, b_qkv, w_proj, b_proj):
    f32 = np.float32
    w_qkv = np.asarray(w_qkv, dtype=f32)
    b_qkv = np.asarray(b_qkv, dtype=f32)
    w_proj = np.asarray(w_proj, dtype=f32)
    b_proj = np.asarray(b_proj, dtype=f32)
    b_v = b_qkv[2 * C :]
    bp2 = b_proj + w_proj @ b_v

    def rearr(wT):  # [C(in), O] -> [P, CCH, O] with in-channel = cc*128 + p
        return np.ascontiguousarray(wT.reshape(CCH, P, -1).transpose(1, 0, 2))

    shared = {
        "w_qkr": np.ascontiguousarray(
            w_qkv[: 2 * C].T.reshape(CCH, P, 2, PAIRS, P)
            .transpose(1, 3, 2, 0, 4)
        ),
        "b_q": np.ascontiguousarray(b_qkv[:C].reshape(CCH, P).T),
        "w_vr": rearr(w_qkv[2 * C :].T),
        "w_pr": rearr(w_proj.T),
        "b_p2": np.ascontiguousarray(bp2.reshape(CCH, P).T),
        "gamma": np.ascontiguousarray(
            np.asarray(gn_gamma, dtype=f32).reshape(CCH, P).T
        ),
        "beta": np.ascontiguousarray(
            np.asarray(gn_beta, dtype=f32).reshape(CCH, P).T
        ),
        "dums": np.full((P, 512), 0.02, __import__("ml_dtypes").bfloat16),
    }
    gsel = np.zeros((P, 8), f32)
    for p in range(P):
        gsel[p, p // GS] = 1.0
    shared["gsel"] = gsel
    shared["gselT"] = np.ascontiguousarray(gsel.T)
    in_maps = []
    for b in range(B):
        m = dict(shared)
        m["x"] = np.ascontiguousarray(np.asarray(x[b], dtype=f32).reshape(C, N))
        in_maps.append(m)
    return in_maps


def kernel(x, gn_gamma, gn_beta, w_qkv, b_qkv, w_proj, b_proj):
    if "nc" not in _CACHE:
        _CACHE["nc"] = build_nc()
    nc = _CACHE["nc"]
    in_maps = make_in_maps(x, gn_gamma, gn_beta, w_qkv, b_qkv, w_proj, b_proj)
    trace = bool(os.environ.get("KERNEL_TRACE"))
    res = run_bass_kernel_spmd(
        nc, in_maps, core_ids=list(range(NCORES)), trace=trace
    )
    _CACHE["last_result"] = res
    out = np.stack([np.asarray(res.results[i]["out"]) for i in range(NCORES)])
    return out.reshape(B, C, 32, 32).astype(np.float32)
